# revision 1
# baseline (speedup 1.0000x reference)
"""Grouped-Query Attention on 8 Trainium2 NeuronCores (Bass/Tile).

Sharding: tensor-parallel across heads. Core c owns KV head c and its 4 query
heads (wq rows [512c:512c+512], wk/wv rows [128c:128c+128]). Attention runs
fully head-local. Attention outputs are exchanged with one AllToAll per batch
so that core c ends up with ALL heads' outputs for its token slice
(batch0 tokens [256c:256c+256) and batch1 tokens likewise); each core then
runs the output projection for its own tokens against the full wo.

Device algorithm (per core, all matmuls bf16 with f32 PSUM accumulation):
 - host pre-transposes/casts: XT (hid, tok) bf16, wqT/wkT/wvT, woT bf16.
 - projections produce qT/kT d-major (feat-in-partitions) and v token-major;
   RoPE applied in f32 straight out of PSUM via DVE (cos/sin tables are host
   inputs; q tables pre-scaled by 1/sqrt(D); sin tables sign-baked so
   rotate_half becomes two partition-shifted multiplies).
 - attention uses transposed scores: scoresT[l,q] = kT_blk^T-over-d @ qT.
   exp on ACT (no max subtraction: scores are O(10) for this data), causal
   masking = multiply by 0/1 bf16 tiles post-exp (diagonal blocks only;
   blocks above the diagonal are skipped, derived from the actual mask on
   host), denominators via DVE accumulation + one ones-matmul partition
   reduce, normalization via reciprocal + gpsimd partition_broadcast.
   outT[d,q] += v_blk^T-over-l @ expT needs no transposes anywhere.
 - O projection: lhsT = attnOT f-major blocks (stationary), rhs = woT tiles.
"""

import sys

for p in ("/opt/trn_rl_repo",):
    if p not in sys.path:
        sys.path.insert(0, p)

import numpy as np
import ml_dtypes

import concourse.bass as bass
import concourse.mybir as mybir
import concourse.tile as tile
from concourse import bacc
from concourse.bass import ts
from concourse.bass_utils import run_bass_kernel_spmd
from concourse.alu_op_type import AluOpType

BF16 = ml_dtypes.bfloat16
F32 = mybir.dt.float32
BF = mybir.dt.bfloat16

HID = 4096
NH = 32          # total query heads
NKV = 8
D = 128
G = NH // NKV    # 4 q heads per kv head / per core
NC = 8
ROPE_THETA = 10000.0


def _build_block_info(attention_mask, S, QC, LB):
    """Classify (b, qchunk, lblock) from the actual additive mask.

    Returns (block_lists, mask_tiles):
      block_lists[b][qc] = list of (lb, mask_tile_idx or -1)
      mask_tiles: float32 array (n, LB, QC): 0/1 multipliers, transposed (l, q).
    Requires a "binary" mask (entries either 0 or <= -30) — true for causal.
    """
    B = attention_mask.shape[0]
    tiles = {}
    order = []
    block_lists = []
    for b in range(B):
        m = attention_mask[b, 0]
        per_b = []
        for qc in range(S // QC):
            qs = qc * QC
            lst = []
            for lb in range(S // LB):
                ls = lb * LB
                sub = m[qs:qs + QC, ls:ls + LB]
                if (sub <= -30.0).all():
                    continue
                if (sub == 0.0).all():
                    lst.append((lb, -1))
                    continue
                ok = ((sub == 0.0) | (sub <= -30.0)).all()
                assert ok, "kernel supports only binary (0 / -inf style) masks"
                pat = (sub.T == 0.0).astype(np.float32)  # (LB, QC)
                key = pat.tobytes()
                if key not in tiles:
                    tiles[key] = len(order)
                    order.append(pat)
                lst.append((lb, tiles[key]))
            per_b.append(lst)
        block_lists.append(per_b)
    if not order:
        order.append(np.ones((LB, QC), np.float32))
    return block_lists, np.stack(order)


def build_program(S, block_lists, n_masks):
    """Emit the SPMD per-core program. Returns the Bass object."""
    B = 2
    NTOK = B * S
    QC, LB = 512, 128
    NTC = NTOK // 512         # token chunks for projections
    NQC = S // QC             # q chunks per batch
    NLBB = S // LB            # l blocks per batch
    TSL = S // NC             # my token slice per batch (256)
    HB = HID // 128           # 32 hidden blocks

    nc = bacc.Bacc()
    xt = nc.declare_dram_parameter("xt", [HID, NTOK], BF, isOutput=False)
    wqt = nc.declare_dram_parameter("wqt", [HID, G * D], BF, isOutput=False)
    wkt = nc.declare_dram_parameter("wkt", [HID, D], BF, isOutput=False)
    wvt = nc.declare_dram_parameter("wvt", [HID, D], BF, isOutput=False)
    wot = nc.declare_dram_parameter("wot", [HID, HID], BF, isOutput=False)
    qcos = nc.declare_dram_parameter("qcos", [D, S], BF, isOutput=False)
    qsin = nc.declare_dram_parameter("qsin", [D, S], BF, isOutput=False)
    kcos = nc.declare_dram_parameter("kcos", [D, S], BF, isOutput=False)
    ksin = nc.declare_dram_parameter("ksin", [D, S], BF, isOutput=False)
    maskt = nc.declare_dram_parameter("maskt", [n_masks, LB, QC], BF, isOutput=False)
    out = nc.declare_dram_parameter("out", [B * TSL, HID], F32, isOutput=True)

    with tile.TileContext(nc) as tc:
        with (
            tc.tile_pool(name="const", bufs=1) as const,
            tc.tile_pool(name="dram", bufs=1, space="DRAM") as dram,
            tc.tile_pool(name="qkv", bufs=1) as qkv,
        ):
            masks = []
            for i in range(n_masks):
                mt = const.tile([LB, QC], BF, tag=f"mask{i}", name=f"mask{i}")
                nc.sync.dma_start(out=mt[:], in_=maskt[i])
                masks.append(mt)
            ones = const.tile([128, 1], F32, tag="ones")
            nc.vector.memset(ones[:], 1.0)
            ones_row = const.tile([1, 128], BF, tag="ones_row")
            nc.vector.memset(ones_row[:], 1.0)

            qT = []
            for h in range(G):
                qT.append(qkv.tile([D, NTOK], BF, tag=f"qT{h}", name=f"qT{h}"))
            kT = qkv.tile([D, NTOK], BF, tag="kT")
            vt = qkv.tile([128, NTOK // 128, D], BF, tag="v")

            a2a_in = []
            a2a_out = []
            for b in range(B):
                a2a_in.append(dram.tile([NC, G * D, TSL], BF, tag=f"a2i{b}", name=f"a2i{b}"))
                a2a_out.append(
                    dram.tile([NC, G * D, TSL], BF, tag=f"a2o{b}",
                              name=f"a2o{b}"))

            # ---------------- phase 1: projections + rope ----------------
            with (
                tc.tile_pool(name="ropec", bufs=1) as ropec,
                tc.tile_pool(name="xtp", bufs=2) as xtp,
                tc.tile_pool(name="wts", bufs=1) as wts,
                tc.tile_pool(name="rtmp", bufs=4) as rtmp,
                tc.tile_pool(name="pqk", bufs=3, space="PSUM") as pqk,
                tc.tile_pool(name="pv", bufs=2, space="PSUM") as pvp,
            ):
                qcos_sb = ropec.tile([D, S], BF, tag="qcos")
                qsin_sb = ropec.tile([D, S], BF, tag="qsin")
                kcos_sb = ropec.tile([D, S], BF, tag="kcos")
                ksin_sb = ropec.tile([D, S], BF, tag="ksin")
                for t, src in ((qcos_sb, qcos), (qsin_sb, qsin),
                               (kcos_sb, kcos), (ksin_sb, ksin)):
                    nc.sync.dma_start(out=t[:], in_=src[:])

                wq_sb = wts.tile([128, HB, G * D], BF, tag="wq")
                nc.sync.dma_start(
                    out=wq_sb[:],
                    in_=wqt.rearrange("(hb p) f -> p hb f", p=128))
                wk_sb = wts.tile([128, HB, D], BF, tag="wk")
                nc.sync.dma_start(
                    out=wk_sb[:],
                    in_=wkt.rearrange("(hb p) f -> p hb f", p=128))
                wv_sb = wts.tile([128, HB, D], BF, tag="wv")
                nc.sync.dma_start(
                    out=wv_sb[:],
                    in_=wvt.rearrange("(hb p) f -> p hb f", p=128))

                def rope(ps, out_sl, cos_sb, sin_sb, tcol):
                    c = cos_sb[:, tcol:tcol + 512]
                    s = sin_sb[:, tcol:tcol + 512]
                    t0 = rtmp.tile([D, 512], F32, tag="r0")
                    t1 = rtmp.tile([D, 512], F32, tag="r1")
                    nc.vector.tensor_tensor(t0[:], ps[:], c, op=AluOpType.mult)
                    nc.vector.tensor_tensor(
                        t1[0:64, :], ps[64:128, :], s[0:64, :], op=AluOpType.mult)
                    nc.vector.tensor_tensor(
                        t1[64:128, :], ps[0:64, :], s[64:128, :], op=AluOpType.mult)
                    nc.vector.tensor_tensor(out_sl, t0[:], t1[:], op=AluOpType.add)

                for tcn in range(NTC):
                    xt_sb = xtp.tile([128, HB, 512], BF, tag="xt")
                    nc.sync.dma_start(
                        out=xt_sb[:],
                        in_=xt[:, ts(tcn, 512)].rearrange(
                            "(hb p) t -> p hb t", p=128))
                    tcol = (tcn * 512) % S
                    for h in range(G):
                        ps = pqk.tile([128, 512], F32, tag="psq")
                        for hb in range(HB):
                            nc.tensor.matmul(
                                ps[:], lhsT=wq_sb[:, hb, ts(h, D)],
                                rhs=xt_sb[:, hb, :],
                                start=(hb == 0), stop=(hb == HB - 1))
                        rope(ps, qT[h][:, ts(tcn, 512)], qcos_sb, qsin_sb, tcol)
                    ps = pqk.tile([128, 512], F32, tag="psq")
                    for hb in range(HB):
                        nc.tensor.matmul(
                            ps[:], lhsT=wk_sb[:, hb, :], rhs=xt_sb[:, hb, :],
                            start=(hb == 0), stop=(hb == HB - 1))
                    rope(ps, kT[:, ts(tcn, 512)], kcos_sb, ksin_sb, tcol)
                    for t4 in range(4):
                        pv = pvp.tile([128, D], F32, tag="psv")
                        for hb in range(HB):
                            nc.tensor.matmul(
                                pv[:], lhsT=xt_sb[:, hb, ts(t4, 128)],
                                rhs=wv_sb[:, hb, :],
                                start=(hb == 0), stop=(hb == HB - 1))
                        nc.scalar.copy(vt[:, tcn * 4 + t4, :], pv[:])

            # ---------------- phase 2: attention + A2A ----------------
            with (
                tc.tile_pool(name="asb", bufs=4) as asb,
                tc.tile_pool(name="sap", bufs=2) as sap,
                tc.tile_pool(name="aop", bufs=3) as aop,
                tc.tile_pool(name="pssc", bufs=2, space="PSUM") as pssc,
                tc.tile_pool(name="pso", bufs=2, space="PSUM") as pso,
                tc.tile_pool(name="pssum", bufs=2, space="PSUM") as pssum,
            ):
                for b in range(B):
                    for h in range(G):
                        for qc in range(NQC):
                            blocks = block_lists[b][qc]
                            nlb = len(blocks)
                            outp = pso.tile([D, 512], F32, tag="outp")
                            sacc = sap.tile([128, 512], F32, tag="sacc")
                            for i, (lb, mi) in enumerate(blocks):
                                scp = pssc.tile([128, 512], F32, tag="scp")
                                nc.tensor.matmul(
                                    scp[:],
                                    lhsT=kT[:, b * S + lb * LB:b * S + (lb + 1) * LB],
                                    rhs=qT[h][:, b * S + qc * QC:b * S + (qc + 1) * QC],
                                    start=True, stop=True)
                                ex = asb.tile([128, 512], BF, tag="ex")
                                nc.scalar.activation(
                                    ex[:], scp[:], mybir.ActivationFunctionType.Exp)
                                if mi >= 0:
                                    nc.vector.tensor_tensor(
                                        ex[:], ex[:], masks[mi][:],
                                        op=AluOpType.mult)
                                if i == 0:
                                    nc.vector.tensor_copy(sacc[:], ex[:])
                                else:
                                    nc.vector.tensor_tensor(
                                        sacc[:], sacc[:], ex[:], op=AluOpType.add)
                                nc.tensor.matmul(
                                    outp[:],
                                    lhsT=vt[:, b * (S // 128) + lb, :],
                                    rhs=ex[:],
                                    start=(i == 0), stop=(i == nlb - 1))
                            sump = pssum.tile([1, 512], F32, tag="sump")
                            nc.tensor.matmul(
                                sump[:], lhsT=ones[:], rhs=sacc[:],
                                start=True, stop=True)
                            rec = asb.tile([1, 512], BF, tag="rec")
                            with nc.allow_low_precision(
                                    reason="softmax denom bf16 broadcast"):
                                nc.vector.reciprocal(rec[:], sump[:])
                            rbc = pssum.tile([128, 512], F32, tag="rbc")
                            nc.tensor.matmul(
                                rbc[:], lhsT=ones_row[:], rhs=rec[:],
                                start=True, stop=True)
                            aot = aop.tile([D, 512], BF, tag="aot")
                            nc.scalar.copy(aot[:], outp[:])
                            ao = aop.tile([D, 512], BF, tag="ao")
                            nc.vector.tensor_tensor(
                                ao[:], aot[:], rbc[:], op=AluOpType.mult)
                            j0 = (qc * QC) // TSL
                            for jj in range(QC // TSL):
                                nc.sync.dma_start(
                                    out=a2a_in[b][j0 + jj, ts(h, D), :],
                                    in_=ao[:, ts(jj, TSL)])
                    nc.gpsimd.collective_compute(
                        "AllToAll", AluOpType.bypass,
                        replica_groups=[list(range(NC))],
                        ins=[a2a_in[b][:]], outs=[a2a_out[b][:]])

            # ---------------- phase 3: O projection ----------------
            with (
                tc.tile_pool(name="afp", bufs=1) as afp,
                tc.tile_pool(name="wop", bufs=2) as wop,
                tc.tile_pool(name="osb", bufs=3) as osb,
                tc.tile_pool(name="pso2", bufs=4, space="PSUM") as pso2,
            ):
                ntok_my = B * TSL  # 512
                attnF = afp.tile([128, HB, ntok_my], BF, tag="attnF")
                for b in range(B):
                    for j in range(NC):
                        for sub in range(G):
                            nc.sync.dma_start(
                                out=attnF[:, j * G + sub,
                                          b * TSL:(b + 1) * TSL],
                                in_=a2a_out[b][j, ts(sub, 128), :])
                for oc in range(HID // 512):
                    wo_sb = wop.tile([128, HB, 512], BF, tag="wo")
                    nc.sync.dma_start(
                        out=wo_sb[:],
                        in_=wot[:, ts(oc, 512)].rearrange(
                            "(fb p) o -> p fb o", p=128))
                    for t4 in range(ntok_my // 128):
                        po = pso2.tile([128, 512], F32, tag="po")
                        for fb in range(HB):
                            nc.tensor.matmul(
                                po[:], lhsT=attnF[:, fb, ts(t4, 128)],
                                rhs=wo_sb[:, fb, :],
                                start=(fb == 0), stop=(fb == HB - 1))
                        ot = osb.tile([128, 512], F32, tag="ot")
                        nc.scalar.copy(ot[:], po[:])
                        nc.sync.dma_start(
                            out=out[ts(t4, 128), ts(oc, 512)], in_=ot[:])
    if not nc.is_finalized():
        nc.finalize()
    return nc


def host_prep(hidden_states, attention_mask, wq, wk, wv, wo, S):
    """Build per-core input maps. Returns (in_maps, B, TSL)."""
    B = hidden_states.shape[0]
    X = np.ascontiguousarray(hidden_states.reshape(B * S, HID))
    XT = np.ascontiguousarray(X.T).astype(BF16)

    inv_freq = 1.0 / (ROPE_THETA ** (np.arange(0, D, 2, dtype=np.float32) / D))
    t = np.arange(S, dtype=np.float32)
    freqs = np.outer(t, inv_freq)
    emb = np.concatenate([freqs, freqs], -1)      # (S, D)
    cos = np.cos(emb).astype(np.float32).T.copy()  # (D, S)
    sin = np.sin(emb).astype(np.float32).T.copy()
    sin_signed = sin.copy()
    sin_signed[:D // 2] *= -1.0
    scale = np.float32(1.0 / np.sqrt(D))
    qcos = (cos * scale).astype(BF16)
    qsin = (sin_signed * scale).astype(BF16)
    kcos, ksin = cos.astype(BF16), sin_signed.astype(BF16)

    block_lists, mask_tiles = _build_block_info(
        np.asarray(attention_mask), S, 512, 128)
    maskt = mask_tiles.astype(BF16)

    woT = np.ascontiguousarray(wo.T).astype(BF16)
    in_maps = []
    for c in range(NC):
        wqT = np.ascontiguousarray(wq[512 * c:512 * (c + 1)].T).astype(BF16)
        wkT = np.ascontiguousarray(wk[128 * c:128 * (c + 1)].T).astype(BF16)
        wvT = np.ascontiguousarray(wv[128 * c:128 * (c + 1)].T).astype(BF16)
        in_maps.append({
            "xt": XT, "wqt": wqT, "wkt": wkT, "wvt": wvT, "wot": woT,
            "qcos": qcos, "qsin": qsin, "kcos": kcos, "ksin": ksin,
            "maskt": maskt,
        })
    return in_maps, block_lists, maskt.shape[0]


_CACHE = {}


def _get_program(key, S, block_lists, n_masks):
    if key not in _CACHE:
        _CACHE[key] = build_program(S, block_lists, n_masks)
    return _CACHE[key]


def kernel(hidden_states, attention_mask, wq, wk, wv, wo, _trace=False):
    B, S, _ = hidden_states.shape
    in_maps, block_lists, n_masks = host_prep(
        hidden_states, attention_mask, wq, wk, wv, wo, S)
    key = (S, n_masks,
           tuple(tuple(tuple(x) for x in bl) for b in block_lists for bl in [b]))
    nc = _get_program(key, S, block_lists, n_masks)
    import time as _time
    _t0 = _time.time()
    try:
        res = run_bass_kernel_spmd(nc, in_maps, list(range(NC)), trace=_trace)
    except ModuleNotFoundError:
        # NTFF profile hook unavailable in this container; run untraced.
        res = run_bass_kernel_spmd(nc, in_maps, list(range(NC)), trace=False)
    _wall_ns = int((_time.time() - _t0) * 1e9)
    TSL = S // NC
    full = np.empty((B, S, HID), np.float32)
    for c in range(NC):
        o = res.results[c]["out"]
        for b in range(B):
            full[b, TSL * c:TSL * (c + 1)] = o[b * TSL:(b + 1) * TSL]
    kernel.last_exec_time_ns = (
        res.exec_time_ns if res.exec_time_ns is not None else _wall_ns)
    kernel.last_results = res
    return full

